# revision 19
# baseline (speedup 1.0000x reference)
"""Trainium2 Bass kernel for CNCAgg (weight-MLP + bmm aggregation + final 1x1 conv).

v2 strategy (8 cores, data-parallel over B=32 with sharded final conv):
  - Per core: 4 batches. WeightNet MLP in bf16 with 2x2 PE-quadrant packing
    (4 batches concurrent). L3 emits transposed (n on partitions) for the bmm.
  - feature is pre-transposed/quantized on host to fp8 e4m3 (halves the
    dominant HBM stream); bmm = fp8 feat x bf16 wgt, accumulated fp32.
  - Final conv weight wf' (512 x 16384, with BN scale and the 1/N feature
    scaling folded in) is sharded 8-way over the contraction dim: each core
    loads a 2048-row slice (2.1 MB instead of 16.8 MB).
  - agg rows are exchanged with AllToAll so each core gets all 32 batches'
    agg for its cw-slice; partial conv outputs (32x512) are exchanged back
    with a second AllToAll and summed on-chip; bias+relu on-chip.
  - The exchange+conv runs in 2 rounds (batches {0,1} then {2,3}); round 0
    overlaps the second half of the feature stream.
"""

import os
import sys

sys.path.insert(0, "/opt/trn_rl_repo")

KSTAGE = int(os.environ.get("KSTAGE", "4"))

import numpy as np
import ml_dtypes

import concourse.bass as bass
from concourse import bacc
import concourse.mybir as mybir
from concourse.bass import ds, ts
from concourse.tile import TileContext
from concourse.bass_utils import run_bass_kernel_spmd

# ---------------------------------------------------------------- constants
B, N, C, OUT, W = 32, 4096, 256, 512, 64
EPS = 1e-5
NCORES = 8
BLOC = B // NCORES            # 4 batches per core
KCW = C * W                   # 16384 contraction dim of final conv
NCH = N // 128                # 32 n-chunks of 128 per batch
KSL = KCW // 128 // NCORES    # 16 k-chunks (of 128) per core slice

F32 = mybir.dt.float32
BF16 = mybir.dt.bfloat16
F8 = mybir.dt.float8e4
NPBF = ml_dtypes.bfloat16
NPF8 = ml_dtypes.float8_e4m3
RELU = mybir.ActivationFunctionType.Relu
ALU = mybir.AluOpType


def build_bass():
    nc = bacc.Bacc("TRN2", target_bir_lowering=False, debug=True,
                   num_devices=NCORES)

    # per-core inputs
    x3_d = nc.dram_tensor("x3p", [128, N], BF16, kind="ExternalInput")
    # featT[b, i, p, jj, c] = feature^T[b, n=512*i+128*jj+p, c] (fp8)
    ft_d = nc.dram_tensor("featT", [BLOC, N // 512, 128, 4, C], F8,
                          kind="ExternalInput")
    w1_d = nc.dram_tensor("w1t", [128, W], BF16, kind="ExternalInput")
    w2_d = nc.dram_tensor("w2t", [128, W], BF16, kind="ExternalInput")
    w3_d = nc.dram_tensor("w3t", [128, W], BF16, kind="ExternalInput")
    b1_d = nc.dram_tensor("b1", [128, 1], F32, kind="ExternalInput")
    b2_d = nc.dram_tensor("b2", [128, 1], F32, kind="ExternalInput")
    b3_d = nc.dram_tensor("b3rep", [128, 8, W], F32, kind="ExternalInput")
    # wfT[kc, p, o] = wf'^T[cw_lin = 2048*core + 128*kc + p, o]  (core slice)
    wf_d = nc.dram_tensor("wfT", [KSL, 128, OUT], BF16, kind="ExternalInput")
    bf_d = nc.dram_tensor("bfrep", [2, 2, OUT], F32, kind="ExternalInput")
    out_d = nc.dram_tensor("out", [2, 2, OUT], F32, kind="ExternalOutput")
    agdbg_d = nc.dram_tensor("aggdbg", [128, KCW // 128, BLOC], F32,
                             kind="ExternalOutput") if KSTAGE >= 90 else None
    ptdbg_d = nc.dram_tensor("partdbg", [2, 16, OUT], F32,
                             kind="ExternalOutput") if KSTAGE >= 90 else None
    rdbg_d = nc.dram_tensor("Rdbg", [2, 2, NCORES, OUT], F32,
                            kind="ExternalOutput") if KSTAGE >= 90 else None
    fdbg_d = nc.dram_tensor("Fdbg", [2, 2, OUT], F32,
                            kind="ExternalOutput") if KSTAGE >= 90 else None

    RG = [list(range(NCORES))]

    with TileContext(nc) as tc:
        with (
            tc.tile_pool(name="const", bufs=1) as cpool,
            tc.tile_pool(name="hbuf", bufs=1) as hpool,
            tc.tile_pool(name="wgt", bufs=2) as wpool,
            tc.tile_pool(name="feat", bufs=20) as fpool,
            tc.tile_pool(name="wfin", bufs=16) as wfpool,
            tc.tile_pool(name="exch", bufs=1) as epool,
            tc.tile_pool(name="osb", bufs=1) as opool,
            tc.tile_pool(name="dram", bufs=1, space="DRAM") as dpool,
            tc.tile_pool(name="ph", bufs=2, space="PSUM") as pph,
            tc.tile_pool(name="pw", bufs=2, space="PSUM") as ppw,
            tc.tile_pool(name="pa", bufs=1, space="PSUM") as ppa,
            tc.tile_pool(name="pf", bufs=1, space="PSUM") as ppf,
        ):
            # ---- constants; w1t + x3 first (L1-critical)
            w1t = cpool.tile([128, W], BF16, tag="w1t")
            nc.sync.dma_start(out=w1t[:], in_=w1_d[:])
            x3 = cpool.tile([128, N], BF16, tag="x3")
            nc.sync.dma_start(out=x3[:], in_=x3_d[:])
            w2t = cpool.tile([128, W], BF16, tag="w2t")
            nc.scalar.dma_start(out=w2t[:], in_=w2_d[:])
            w3t = cpool.tile([128, W], BF16, tag="w3t")
            nc.scalar.dma_start(out=w3t[:], in_=w3_d[:])
            b1t = cpool.tile([128, 1], F32, tag="b1")
            nc.scalar.dma_start(out=b1t[:], in_=b1_d[:])
            b2t = cpool.tile([128, 1], F32, tag="b2")
            nc.scalar.dma_start(out=b2t[:], in_=b2_d[:])
            b3t = cpool.tile([128, 8, W], F32, tag="b3")
            nc.scalar.dma_start(out=b3t[:], in_=b3_d[:])
            bft = cpool.tile([2, 2, OUT], F32, tag="bf")
            nc.scalar.dma_start(out=bft[:], in_=bf_d[:])
            # agg_s[p, kk, b]: agg[cw_lin = 128*kk + p, b] (bf16, unscaled)
            agg_s = cpool.tile([128, KCW // 128, BLOC], BF16, tag="aggs")
            zeros = cpool.tile([128, 2, 256], BF16, tag="zeros")
            nc.vector.memset(zeros[:], 0.0)

            # ---- feature tiles: (128, 4, C) fp8, 8 per batch, streamed
            ft_tiles = {}

            def load_ft(b, i):
                ft = fpool.tile([128, 4, C], F8, tag="ft", name=f"ft{b}_{i}")
                nc.sync.dma_start(out=ft[:], in_=ft_d[b, i])
                ft_tiles[(b, i)] = ft

            # stream order: pair (0,1) interleaved, then pair (2,3)
            for i in range(N // 512):
                load_ft(0, i)
                load_ft(1, i)

            # ---- wf tiles (for the sharded conv), behind featT in priority
            wf_tiles = []

            def load_wf(kc):
                wt = wfpool.tile([128, OUT], BF16, tag="wf", name=f"wf{kc}")
                nc.scalar.dma_start(out=wt[:], in_=wf_d[kc])
                wf_tiles.append(wt)

            for kc in range(KSL):
                load_wf(kc)

            # ---- L1: 3 -> 64, 4 batches on PE quadrants
            # x3 strips: batch b on partitions 32b..32b+2
            # quadrants: b0 (0,0)->ps[0:64, 0]; b1 (32,0)->ps[0:64, 1];
            #            b2 (64,64)->ps[64:128, 0]; b3 (96,64)->ps[64:128, 1]
            # h layout h[p, half, n]: half0 = {b0 low, b2 high}, half1 = {b1, b3}
            h1 = hpool.tile([128, 2, N], BF16, tag="h1")
            h2 = hpool.tile([128, 2, N], BF16, tag="h2")
            # concurrent row-tiled matmuls MUST land in different PSUM banks:
            # even batches -> bank A, odd batches -> bank B (each a full bank)
            FCH = 512
            for i in range(N // FCH):
                psA = pph.tile([128, FCH], F32, tag="hpsA")
                psB = pph.tile([128, FCH], F32, tag="hpsB")
                for b in range(BLOC):
                    row = 32 * b
                    col = 64 * (b // 2)
                    ps = psA if b % 2 == 0 else psB
                    nc.tensor.matmul(
                        ps[ds(col, W), :],
                        lhsT=w1t[ds(row, 3), :],
                        rhs=x3[ds(row, 3), ds(i * FCH, FCH)],
                        start=True, stop=True,
                        tile_position=(row, col), skip_group_check=True,
                    )
                nc.scalar.activation(
                    h1[:, 0, ds(i * FCH, FCH)], psA[:], RELU, bias=b1t[:]
                )
                nc.vector.scalar_tensor_tensor(
                    h1[:, 1, ds(i * FCH, FCH)],
                    in0=psB[:], scalar=b1t[:], in1=zeros[:],
                    op0=ALU.add, op1=ALU.max,
                )

            # ---- L2: 64 -> 64, 4 batches on quadrants
            # b0 (0,0) rhs h1[0:64,0] -> ps[0:64, 0]
            # b2 (64,0) rhs h1[64:128,0] -> ps[0:64, 1]
            # b1 (0,64) rhs h1[0:64,1] -> ps[64:128, 0]
            # b3 (64,64) rhs h1[64:128,1] -> ps[64:128, 1]
            # h2 layout: h2[0:64,0]=b0, h2[64:128,0]=b1, h2[0:64,1]=b2, h2[64:128,1]=b3
            # i.e. batch b -> partitions 64*(b%2), free-half b//2
            # L1 emits batch b at h1[64*(b//2) rows, free-half b%2].
            # L2 quadrants (srow = h1 strip, col = out partitions):
            #   b0 (0,0)->A[0:64], b1 (0,64)->A[64:128],
            #   b2 (64,0)->B[0:64], b3 (64,64)->B[64:128]
            # => h2[:,0] = [b0|b1], h2[:,1] = [b2|b3]:
            #    batch b at h2[64*(b%2) rows, free-half b//2] (what L3 wants)
            for i in range(N // FCH):
                psA = pph.tile([128, FCH], F32, tag="hpsA")
                psB = pph.tile([128, FCH], F32, tag="hpsB")
                for b in range(BLOC):
                    srow = 64 * (b // 2)     # h1 partition strip of batch b
                    col = 64 * (b % 2)
                    ps = psA if b < 2 else psB
                    nc.tensor.matmul(
                        ps[ds(col, W), :],
                        lhsT=w2t[ds(srow, W), :],
                        rhs=h1[ds(srow, W), b % 2, ds(i * FCH, FCH)],
                        start=True, stop=True,
                        tile_position=(srow, col), skip_group_check=True,
                    )
                nc.vector.scalar_tensor_tensor(
                    h2[:, 0, ds(i * FCH, FCH)],
                    in0=psA[:], scalar=b2t[:], in1=zeros[:],
                    op0=ALU.add, op1=ALU.max,
                )
                nc.scalar.activation(
                    h2[:, 1, ds(i * FCH, FCH)], psB[:], RELU, bias=b2t[:]
                )

            # ---- L3 (transposed out): per batch, wgt[b] = (n x w) bf16
            # batch b: h2 strip rows 64*(b%2), free-half b//2
            wgt_tiles = [None] * BLOC

            def l3_group(b, j):
                """8 n-chunks (j*8 .. j*8+8) of batch b's transposed L3."""
                row = 64 * (b % 2)
                if wgt_tiles[b] is None:
                    wgt_tiles[b] = wpool.tile([128, NCH, W], BF16, tag="wgt",
                                              name=f"wgt{b}")
                wgt = wgt_tiles[b]
                pwg = ppw.tile([128, 8, W], F32, tag="wps")
                for jj in range(8):
                    i = j * 8 + jj
                    nc.tensor.matmul(
                        pwg[:, jj, :],
                        lhsT=h2[ds(row, W), b // 2, ds(i * 128, 128)],
                        rhs=w3t[ds(row, W), :],
                        start=True, stop=True,
                        tile_position=(row, 0), skip_group_check=True,
                    )
                nc.vector.tensor_add(pwg[:], pwg[:], b3t[:])
                nc.scalar.activation(wgt[:, ts(j, 8), :], pwg[:], RELU)

            # ---- bmm for a batch pair (be, bo) = (2q, 2q+1), col-tiled 2x
            # b even -> col 0 (pa[0:64]), b odd -> col 64 (pa[64:128])
            def bmm_pair(q, interleave=None):
                be, bo = 2 * q, 2 * q + 1
                pa = ppa.tile([128, C], F32, tag="aps")
                for i in range(N // 512):
                    for b, col in ((be, 0), (bo, 64)):
                        if (b, i) not in ft_tiles:
                            load_ft(b, i)
                        ftile = ft_tiles[(b, i)]
                        wgt = wgt_tiles[b]
                        for jj in range(4):
                            ch = 4 * i + jj
                            nc.tensor.matmul(
                                pa[ds(col, W), :],
                                lhsT=wgt[:, ch, :],
                                rhs=ftile[:, jj, :],
                                start=(ch == 0), stop=(ch == NCH - 1),
                                tile_position=(0, col),
                                skip_group_check=True,
                            )
                    if interleave is not None:
                        interleave(i)
                # shuffle into agg_s: agg_s[64*(c%2)+w, c//2, b] = pa[wrow, c]
                for b, base in ((be, 0), (bo, 64)):
                    pav = pa[ds(base, W), :].rearrange("w (k two) -> w two k",
                                                       two=2)
                    nc.vector.tensor_copy(agg_s[0:W, :, b], pav[:, 0, :])
                    nc.vector.tensor_copy(agg_s[W:128, :, b], pav[:, 1, :])

            # L3 for pair 0 up front; pair 1's L3 interleaves into bmm(0)
            for j in range(4):
                l3_group(0, j)
            for j in range(4):
                l3_group(1, j)

            def inter0(step):
                # prefetch pair-1 features + compute pair-1 L3 inside bmm(0)
                if step < 4:
                    l3_group(2, step)
                else:
                    l3_group(3, step - 4)

            # stream pair-1 features behind pair 0
            for i in range(N // 512):
                load_ft(2, i)
                load_ft(3, i)

            bmm_pair(0, interleave=inter0)

            # ---------------- round 0 exchange (batches 0,1 of every core)
            a1_in = [None, None]
            a1_out = [None, None]
            a2_in = [None, None]
            a2_out = [None, None]
            for r in range(2):
                a1_in[r] = dpool.tile([NCORES, 128, KSL, 2], BF16,
                                      name=f"a1in{r}")
                a1_out[r] = dpool.tile([NCORES, 128, KSL, 2], BF16,
                                       name=f"a1out{r}")
                a2_in[r] = dpool.tile([NCORES * 2, OUT], F32, name=f"a2in{r}")
                a2_out[r] = dpool.tile([NCORES, 2, OUT], F32,
                                       name=f"a2out{r}")

            def exchange_round(r):
                """Bounce agg (local batches 2r, 2r+1) out + A2A."""
                for j in range(NCORES):
                    nc.scalar.dma_start(
                        out=a1_in[r][j],
                        in_=agg_s[:, ds(KSL * j, KSL), ds(2 * r, 2)],
                    )
                nc.gpsimd.collective_compute(
                    "AllToAll", ALU.bypass, replica_groups=RG,
                    ins=[a1_in[r][:].opt()], outs=[a1_out[r][:].opt()],
                )

            def conv_round(r):
                """Sharded conv for round r + partial exchange + epilogue."""
                # A[p, k, (rk, b)] = agg of (core rk, batch 2r+b), my slice
                A = epool.tile([128, KSL, 2 * NCORES], BF16, name=f"A{r}")
                for rr in range(NCORES):
                    nc.scalar.dma_start(out=A[:, :, ds(2 * rr, 2)],
                                        in_=a1_out[r][rr])
                pf = ppf.tile([128, OUT], F32, tag="fps")
                for k in range(KSL):
                    g = k % 4
                    nc.tensor.matmul(
                        pf[ds(32 * g, 16), :],
                        lhsT=A[:, k, :],
                        rhs=wf_tiles[k][:],
                        start=(k < 4), stop=(k >= KSL - 4),
                        tile_position=(0, 32 * g),
                        skip_group_check=True,
                    )
                # combine the 4 column-group partials -> (16, OUT)
                part = opool.tile([16, OUT], F32, name=f"part{r}")
                nc.vector.tensor_copy(part[:], pf[ds(0, 16), :])
                for g in range(1, 4):
                    nc.vector.tensor_add(part[:], part[:], pf[ds(32 * g, 16), :])
                nc.scalar.dma_start(out=a2_in[r][:], in_=part[:])
                if KSTAGE >= 90:
                    nc.sync.dma_start(out=ptdbg_d[r], in_=part[:])
                nc.gpsimd.collective_compute(
                    "AllToAll", ALU.bypass, replica_groups=RG,
                    ins=[a2_in[r][:].opt()], outs=[a2_out[r][:].opt()],
                )

            def epilogue_round(r, out_sb):
                # R[b, rk, o]: rank rk's partial for my local batch 2r+b
                R = epool.tile([2, NCORES, OUT], F32, name=f"R{r}")
                nc.sync.dma_start(
                    out=R[:], in_=a2_out[r][:].rearrange("r b o -> b r o")
                )
                # fold the 8 rank-partials along the free axis
                if KSTAGE >= 90:
                    nc.sync.dma_start(out=rdbg_d[r], in_=R[:])
                F1 = epool.tile([2, 4, OUT], F32, name=f"F1_{r}")
                nc.vector.tensor_add(F1[:], R[:, 0:4, :], R[:, 4:8, :])
                F2 = epool.tile([2, 2, OUT], F32, name=f"F2_{r}")
                nc.vector.tensor_add(F2[:], F1[:, 0:2, :], F1[:, 2:4, :])
                F3 = epool.tile([2, 1, OUT], F32, name=f"F3_{r}")
                nc.vector.tensor_add(F3[:], F2[:, 0:1, :], F2[:, 1:2, :])
                nc.vector.tensor_add(F3[:], F3[:], bft[:, ds(r, 1), :])
                if KSTAGE >= 90:
                    nc.sync.dma_start(out=fdbg_d[r], in_=F3[:, 0, :])
                G = epool.tile([2, OUT], F32, name=f"G{r}")
                nc.scalar.activation(G[:], F3[:, 0, :], RELU)
                nc.sync.dma_start(out=out_d[r], in_=G[:])

            out_sb = opool.tile([2, 2, OUT], F32, tag="out")
            if KSTAGE >= 90:
                agf = opool.tile([128, KCW // 128, BLOC], F32, tag="agf")
                nc.vector.tensor_copy(agf[:], agg_s[:])
                nc.sync.dma_start(out=agdbg_d[:], in_=agf[:])
            if KSTAGE >= 2:
                exchange_round(0)

            # ---------------- pair 1 bmm while round-0 exchange flies
            bmm_pair(1)
            if KSTAGE >= 2:
                exchange_round(1)

            if KSTAGE >= 3:
                conv_round(0)
                if KSTAGE >= 4:
                    epilogue_round(0, out_sb)
                conv_round(1)
                if KSTAGE >= 4:
                    epilogue_round(1, out_sb)
            if KSTAGE < 4:
                nc.vector.memset(out_sb[:], 0.0)
                nc.sync.dma_start(out=out_d[:],
                                  in_=out_sb[:].rearrange("b r o -> r b o"))

    nc.compile()
    return nc


_NC_CACHE = None


def _get_nc():
    global _NC_CACHE
    if _NC_CACHE is None:
        _NC_CACHE = build_bass()
    return _NC_CACHE


def _fold_bn(w, b, g, be, m, v):
    """Fold eval-mode BN into conv weight/bias: y = diag(s) W x + (s*(b-m)+be)."""
    s = (g / np.sqrt(v + EPS)).astype(np.float64)
    wp = (w.astype(np.float64) * s[:, None]).astype(np.float32)
    bp = (s * (b.astype(np.float64) - m) + be).astype(np.float32)
    return wp, bp


def prep_inputs(xyz, feature, w1, b1, g1, be1, m1, v1,
                w2, b2, g2, be2, m2, v2,
                w3, b3, g3, be3, m3, v3,
                wf, bf, gf, bef, mf, vf):
    """Host-side prep: BN folding, transposes, per-core sharding."""
    w1p, b1p = _fold_bn(w1, b1, g1, be1, m1, v1)
    w2p, b2p = _fold_bn(w2, b2, g2, be2, m2, v2)
    w3p, b3p = _fold_bn(w3, b3, g3, be3, m3, v3)
    wfp, bfp = _fold_bn(wf, bf, gf, bef, mf, vf)
    # 1/N feature scaling folded into the final conv weight (keeps wgt and
    # agg in healthy fp8/bf16 ranges)
    wfp = (wfp / N).astype(np.float32)

    # w1t strips: w1p.T at partition rows {0,32,64,96}
    w1t = np.zeros((128, W), dtype=np.float32)
    for b in range(BLOC):
        w1t[32 * b:32 * b + 3] = w1p.T
    # wfT permuted rows: cw_lin(c, w) = 128*(c//2) + 64*(c%2) + w
    cw = np.arange(KCW)
    c_idx = cw // W
    w_idx = cw % W
    cw_lin = 128 * (c_idx // 2) + 64 * (c_idx % 2) + w_idx
    wfT_perm = np.empty((KCW, OUT), dtype=np.float32)
    wfT_perm[cw_lin] = wfp.T          # row cw_lin <- wf'[:, c*64+w]

    shared = {
        "w1t": w1t.astype(NPBF),
        "w2t": np.ascontiguousarray(np.tile(w2p.T, (2, 1))).astype(NPBF),
        "w3t": np.ascontiguousarray(np.tile(w3p.T, (2, 1))).astype(NPBF),
        "b1": np.tile(b1p, 2).reshape(128, 1).astype(np.float32),
        "b2": np.tile(b2p, 2).reshape(128, 1).astype(np.float32),
        "b3rep": np.tile(b3p, (128, 8, 1)).astype(np.float32),
        "bfrep": np.tile(bfp, (2, 2, 1)).astype(np.float32),
    }
    in_maps = []
    for core in range(NCORES):
        xs = xyz[core * BLOC:(core + 1) * BLOC]        # (4, 4096, 3)
        x3p = np.zeros((128, N), dtype=np.float32)
        for b in range(BLOC):
            x3p[32 * b:32 * b + 3] = xs[b].T
        fs = feature[core * BLOC:(core + 1) * BLOC]    # (4, 256, 4096)
        ftT = fs.transpose(0, 2, 1)                    # (4, 4096, 256)
        # [b, i, p, jj, c] = featT[b, 512*i + 128*jj + p, c]
        ftT = ftT.reshape(BLOC, N // 512, 4, 128, C).transpose(0, 1, 3, 2, 4)
        wfT = wfT_perm[2048 * core:2048 * (core + 1)].reshape(KSL, 128, OUT)
        in_maps.append({
            "x3p": x3p.astype(NPBF),
            "featT": np.ascontiguousarray(ftT).astype(NPF8),
            "wfT": np.ascontiguousarray(wfT).astype(NPBF),
            **shared,
        })
    return in_maps


def _run(inputs, trace=False):
    inputs = {k: np.asarray(v) for k, v in inputs.items()}
    nc = _get_nc()
    in_maps = prep_inputs(
        inputs["xyz"], inputs["feature"],
        inputs["w1"], inputs["b1"], inputs["g1"], inputs["be1"], inputs["m1"], inputs["v1"],
        inputs["w2"], inputs["b2"], inputs["g2"], inputs["be2"], inputs["m2"], inputs["v2"],
        inputs["w3"], inputs["b3"], inputs["g3"], inputs["be3"], inputs["m3"], inputs["v3"],
        inputs["wf"], inputs["bf"], inputs["gf"], inputs["bef"], inputs["mf"], inputs["vf"],
    )
    res = run_bass_kernel_spmd(
        nc, in_maps, core_ids=list(range(NCORES)), trace=trace,
        trace_cores=list(range(NCORES)) if trace else None,
    )
    outs = [np.asarray(res.results[i]["out"]).reshape(BLOC, OUT)
            for i in range(NCORES)]
    full = np.concatenate(outs, axis=0).astype(np.float32)             # (32, 512)
    return full.reshape(B, OUT, 1), res


def kernel(**inputs):
    return _run(inputs, trace=False)[0]
